# revision 1
# baseline (speedup 1.0000x reference)
"""Trainium2 Bass kernel: conv/pool front-end + LSTM + log_softmax.

Strategy (8 NeuronCores, no cross-core communication):
  - Time-shard T=8192 into 8 blocks of 1024, each core processing a
    1152-row window (64-row warm-up prefix discarded on the host; the
    LSTM fixed-point contraction kills the wrong-boundary error within
    ~40 steps, validated offline to ~1e-4..1e-5).
  - The sequential LSTM is solved by Jacobi fixed-point iteration over
    the whole block: each sweep is a batched matmul G = W_hh @ H_shift
    (hidden-on-partitions layout, so the time shift is a free-dim
    offset), gates via ScalarE with the bias folded in, and the cell
    recurrence c_t = f_t*c_{t-1} + u_t solved EXACTLY per sweep by the
    hardware prefix-scan (tensor_tensor_scan). 12 sweeps reach the
    bf16 noise floor (~2e-3 max|dH|).
  - Conv front-end as 9 dh-shifted matmuls over a PE-transposed
    feature tile; maxpool via partition-base-aligned DVE maxes.
"""

import numpy as np
import ml_dtypes

T = 8192
D = 106
H = 512
PHONE = 48
NCORES = 8
BLK = 1024          # rows owned per core
OV = 64             # warm-up prefix rows
L = BLK + OV        # 1088 rows computed per core
LIN = L + 8         # input rows incl. conv halo (+-4)
NSWEEPS = 6
SKIP = set()  # debug: subset of {'tp','conv','gx','out'}
NCH = [(0, 384), (384, 768), (768, 1088)]  # time chunks (free dim)
PSW = 384           # psum tile width for chunked phases

bf16 = ml_dtypes.bfloat16

_cache = {}


def _host_pack(conv_w, conv_b, w_ih, w_hh, b_ih, b_hh, out_w, out_b):
    """Pure weight repacking/quantization (host-side, one-time)."""
    key = hash((conv_w.tobytes(), w_ih.tobytes(), w_hh.tobytes(), b_ih.tobytes(),
                b_hh.tobytes(), out_w.tobytes(), out_b.tobytes(), conv_b.tobytes()))
    if _cache.get("pack_key") == key:
        return _cache["pack"]
    # conv weight: 7 M-chunks of 128 cols = [d0:32 pairs][d1][d2][pad 32]
    # pair p = c*21 + w' (reference feature order), block i covers pairs
    # [32i, 32i+32); col of chunk i = delta*32 + j for pair 32i+j.
    WA = np.zeros((9, 67, 7 * 128), np.float32)
    p_all = np.arange(210)
    c_all, wp_all = np.divmod(p_all, 21)
    i_all, j_all = np.divmod(p_all, 32)
    for d in range(3):
        w_all = 3 * wp_all + d                      # [210]
        col = 128 * i_all + 32 * d + j_all          # [210]
        for dv in range(5):
            # WA[dh, w+dv, col] = conv_w[c, 0, dh, dv] for all dh at once
            WA[:, w_all + dv, col] = conv_w[c_all, 0, :, dv].T
    # w_ih split: A = features 0..127, B = 128..209 (pool blocks 4..6 at
    # bases 0/32/64, junk rows 82..95 zero), C = mfcc 210..248
    wihA = w_ih[:, 0:128].T.copy()
    wihB = np.zeros((96, 2048), np.float32)
    wihB[0:64] = w_ih[:, 128:192].T
    wihB[64:82] = w_ih[:, 192:210].T
    wihC = w_ih[:, 210:249].T.copy()
    # effective gate bias: b_ih + b_hh + w_ih[:, :210] @ expand(conv_b)
    # (conv bias commutes with the maxpool)
    cb = np.repeat(conv_b, 21)
    beff = b_ih + b_hh + w_ih[:, :210] @ cb
    pack = {
        "convW": WA.astype(bf16),
        "wihA": wihA.astype(bf16),
        "wihB": wihB.astype(bf16),
        "wihC": wihC.astype(bf16),
        "whhT": np.ascontiguousarray(w_hh.T.reshape(4, 128, 2048)).astype(bf16),
        "beff": np.ascontiguousarray(beff.reshape(16, 128).T).astype(np.float32),
        "outwT": np.ascontiguousarray(out_w.T.reshape(4, 128, PHONE)).astype(bf16),
        "outb": out_b.reshape(1, PHONE).astype(bf16),
        "identb": np.eye(128, dtype=np.float32).astype(bf16),
        "identf": np.eye(128, dtype=np.float32),
    }
    _cache["pack_key"] = key
    _cache["pack"] = pack
    return pack


def _build_nc():
    import concourse.bacc as bacc
    import concourse.tile as tile
    import concourse.mybir as mybir

    dt = mybir.dt
    AF = mybir.ActivationFunctionType
    ALU = mybir.AluOpType

    nc = bacc.Bacc(None, target_bir_lowering=False)

    inp = nc.declare_dram_parameter("inp", [LIN, D], dt.float32, isOutput=False)
    h0c0 = nc.declare_dram_parameter("h0c0", [128, 8], dt.float32, isOutput=False)
    convW = nc.declare_dram_parameter("convW", [9, 67, 896], dt.bfloat16, isOutput=False)
    wihA = nc.declare_dram_parameter("wihA", [128, 2048], dt.bfloat16, isOutput=False)
    wihB = nc.declare_dram_parameter("wihB", [96, 2048], dt.bfloat16, isOutput=False)
    wihC = nc.declare_dram_parameter("wihC", [39, 2048], dt.bfloat16, isOutput=False)
    whhT = nc.declare_dram_parameter("whhT", [4, 128, 2048], dt.bfloat16, isOutput=False)
    beff = nc.declare_dram_parameter("beff", [128, 16], dt.float32, isOutput=False)
    outwT = nc.declare_dram_parameter("outwT", [4, 128, PHONE], dt.bfloat16, isOutput=False)
    outb = nc.declare_dram_parameter("outb", [1, PHONE], dt.bfloat16, isOutput=False)
    identb = nc.declare_dram_parameter("identb", [128, 128], dt.bfloat16, isOutput=False)
    identf = nc.declare_dram_parameter("identf", [128, 128], dt.float32, isOutput=False)
    out = nc.declare_dram_parameter("out", [L, PHONE], dt.float32, isOutput=True)

    # gate order (PyTorch): i, f, g, o -> m-chunk m//4 gives gate type
    def gate_func(m):
        return AF.Tanh if 8 <= m < 12 else AF.Sigmoid

    with tile.TileContext(nc) as tc:
        with tc.tile_pool(name="persist", bufs=1) as pp:
            # ---- persistent tiles ----
            featT = pp.tile([67, LIN], dt.bfloat16, tag="featT", name="featT")
            mfccT = pp.tile([39, LIN], dt.bfloat16, tag="mfccT", name="mfccT")
            tileA = pp.tile([128, L], dt.bfloat16, tag="tileA", name="tileA")
            tileB = pp.tile([96, L], dt.bfloat16, tag="tileB", name="tileB")
            gx = [pp.tile([128, L], dt.bfloat16, tag=f"gx{m}", name=f"gx{m}") for m in range(16)]
            Ht = [pp.tile([128, L + 1], dt.bfloat16, tag=f"H{k}", name=f"H{k}") for k in range(4)]
            Ct = [pp.tile([128, L], dt.float32, tag=f"C{k}", name=f"C{k}") for k in range(4)]
            wA9 = [pp.tile([67, 896], dt.bfloat16, tag=f"wA9_{dh}", name=f"wA9_{dh}") for dh in range(9)]
            wiA = pp.tile([128, 2048], dt.bfloat16, tag="wiA", name="wiA")
            wiB = pp.tile([96, 2048], dt.bfloat16, tag="wiB", name="wiB")
            wiC = pp.tile([39, 2048], dt.bfloat16, tag="wiC", name="wiC")
            whh = [pp.tile([128, 2048], dt.bfloat16, tag=f"whh{k}", name=f"whh{k}") for k in range(4)]
            bft = pp.tile([128, 16], dt.float32, tag="bft", name="bft")
            owT = [pp.tile([128, PHONE], dt.bfloat16, tag=f"owT{k}", name=f"owT{k}") for k in range(4)]
            obT = pp.tile([1, PHONE], dt.bfloat16, tag="obT", name="obT")
            idb = pp.tile([128, 128], dt.bfloat16, tag="idb", name="idb")
            idf = pp.tile([128, 128], dt.float32, tag="idf", name="idf")
            hc = pp.tile([128, 8], dt.float32, tag="hc", name="hc")
            ones1 = pp.tile([1, 128], dt.bfloat16, tag="ones1", name="ones1")

            _dmas = ([(idf, identf), (hc, h0c0), (bft, beff), (obT, outb), (idb, identb)]
                     + [(wA9[dh], convW[dh]) for dh in range(9)]
                     + [(wiA, wihA), (wiB, wihB), (wiC, wihC)]
                     + [(whh[k], whhT[k]) for k in range(4)]
                     + [(owT[k], outwT[k]) for k in range(4)])
            for _i, (dst, src) in enumerate(_dmas):
                # weights on the gpsimd queue; sync queue stays free for the
                # input chunks (critical path: transpose -> conv -> gates)
                (nc.gpsimd if _i % 4 else nc.sync).dma_start(dst[:], src[:])
            nc.gpsimd.memset(tileB[:], 0.0)
            nc.gpsimd.memset(ones1[:], 1.0)
            for k in range(4):
                nc.vector.tensor_copy(Ht[k][:, 0:1], hc[:, k:k + 1])

            # ---- input transpose (chunks of <=128 rows) ----
            tchunks = []
            _p = 0
            while _p < LIN:
                _w = min(122, LIN - _p)
                tchunks.append((_p, _w))
                _p += _w
            if 'tp' in SKIP: tchunks = []
            with tc.tile_pool(name="tp_in", bufs=3) as tin, \
                 tc.tile_pool(name="tp_ps", bufs=2, space="PSUM") as tps:
                for (p0, cw) in tchunks:
                    xt = tin.tile([122, D], dt.float32, tag="xt", name="xt")
                    nc.sync.dma_start(xt[0:cw, :], inp[p0:p0 + cw, :])
                    pm = tps.tile([39, 122], dt.float32, tag="pm", name="pm")
                    nc.tensor.transpose(pm[:, 0:cw], xt[0:cw, 0:39], idf[0:cw, 0:cw])
                    nc.vector.tensor_copy(mfccT[:, p0:p0 + cw], pm[:, 0:cw])
                    pf = tps.tile([67, 122], dt.float32, tag="pf", name="pf")
                    nc.tensor.transpose(pf[:, 0:cw], xt[0:cw, 39:106], idf[0:cw, 0:cw])
                    nc.vector.tensor_copy(featT[:, p0:p0 + cw], pf[:, 0:cw])

            # ---- conv + maxpool ----
            with tc.tile_pool(name="cv_ps", bufs=3, space="PSUM") as cps, \
                 tc.tile_pool(name="cv_sb", bufs=3) as csb:
                for i in range(0 if 'conv' in SKIP else 7):
                    for (n0, n1) in NCH:
                        w = n1 - n0
                        ps = cps.tile([128, PSW], dt.float32, tag="cvps", name="cvps")
                        for dh in range(9):
                            nc.tensor.matmul(
                                ps[:, 0:w],
                                wA9[dh][:, 128 * i:128 * (i + 1)],
                                featT[:, n0 + dh:n1 + dh],
                                start=(dh == 0), stop=(dh == 8))
                        if i < 4:
                            dst = tileA[32 * i:32 * (i + 1), n0:n1]
                            rows = 32
                        elif i < 6:
                            dst = tileB[32 * (i - 4):32 * (i - 3), n0:n1]
                            rows = 32
                        else:
                            dst = tileB[64:82, n0:n1]
                            rows = 18
                        tmp = csb.tile([32, PSW], dt.float32, tag="pooltmp", name="pooltmp")
                        nc.vector.tensor_copy(tmp[0:rows, 0:w], ps[0:rows, 0:w])
                        nc.vector.tensor_max(tmp[0:rows, 0:w], tmp[0:rows, 0:w],
                                             ps[32:32 + rows, 0:w])
                        nc.vector.tensor_max(dst, tmp[0:rows, 0:w], ps[64:64 + rows, 0:w])

            # ---- gates_x = w_ih @ lstm_in.T  (bf16, no bias) ----
            with tc.tile_pool(name="gx_ps", bufs=4, space="PSUM") as gps:
                for m in range(0 if 'gx' in SKIP else 16):
                    for (n0, n1) in NCH:
                        w = n1 - n0
                        ps = gps.tile([128, PSW], dt.float32, tag="gxps", name="gxps")
                        nc.tensor.matmul(ps[:, 0:w], wiA[:, 128 * m:128 * (m + 1)],
                                         tileA[:, n0:n1], start=True, stop=False)
                        nc.tensor.matmul(ps[:, 0:w], wiB[:, 128 * m:128 * (m + 1)],
                                         tileB[:, n0:n1], start=False, stop=False)
                        nc.tensor.matmul(ps[:, 0:w], wiC[:, 128 * m:128 * (m + 1)],
                                         mfccT[:, 4 + n0:4 + n1], start=False, stop=True)
                        nc.vector.tensor_copy(gx[m][:, n0:n1], ps[:, 0:w])

            # ---- Jacobi sweeps ----
            with tc.tile_pool(name="sw_ps", bufs=8, space="PSUM") as sps, \
                 tc.tile_pool(name="sw_sb", bufs=6) as ssb:
                for s in range(NSWEEPS):
                    for (n0, n1) in NCH:
                        w = n1 - n0
                        fo = {}
                        us = {}
                        for k in range(4):
                            gt = {}
                            for m in (k, 8 + k, 4 + k, 12 + k):
                                g = ssb.tile([128, PSW], dt.float32, tag=f"g{m // 4}", name=f"g{m // 4}")
                                if s == 0:
                                    nc.scalar.activation(
                                        g[:, 0:w], gx[m][:, n0:n1], gate_func(m),
                                        bias=bft[:, m:m + 1])
                                else:
                                    ps = sps.tile([128, PSW], dt.float32, tag="swps", name="swps")
                                    for k2 in range(4):
                                        nc.tensor.matmul(
                                            ps[:, 0:w],
                                            whh[k2][:, 128 * m:128 * (m + 1)],
                                            Ht[k2][:, n0:n1],
                                            start=(k2 == 0), stop=False)
                                    nc.tensor.matmul(ps[:, 0:w], idb[:],
                                                     gx[m][:, n0:n1], start=False, stop=True)
                                    nc.scalar.activation(
                                        g[:, 0:w], ps[:, 0:w], gate_func(m),
                                        bias=bft[:, m:m + 1])
                                gt[m] = g
                            u = ssb.tile([128, PSW], dt.float32, tag=f"u{k}", name=f"u{k}")
                            nc.vector.tensor_mul(u[:, 0:w], gt[k][:, 0:w], gt[8 + k][:, 0:w])
                            us[k] = u
                            fo[k] = (gt[4 + k], gt[12 + k])
                        for k in range(4):
                            init = hc[:, 4 + k:5 + k] if n0 == 0 else Ct[k][:, n0 - 1:n0]
                            nc.vector.tensor_tensor_scan(
                                Ct[k][:, n0:n1], fo[k][0][:, 0:w], us[k][:, 0:w],
                                init, ALU.mult, ALU.add)
                            tc_t = ssb.tile([128, PSW], dt.float32, tag="tc", name="tc")
                            nc.scalar.activation(tc_t[:, 0:w], Ct[k][:, n0:n1], AF.Tanh)
                            nc.vector.tensor_mul(Ht[k][:, 1 + n0:1 + n1],
                                                 fo[k][1][:, 0:w], tc_t[:, 0:w])

            # ---- output projection + log_softmax ----
            with tc.tile_pool(name="o_ps", bufs=3, space="PSUM") as ops, \
                 tc.tile_pool(name="o_sb", bufs=3) as osb:
                for c in range(0 if 'out' in SKIP else (L + 127) // 128):
                    cw = min(128, L - 128 * c)
                    ps = ops.tile([128, PHONE], dt.float32, tag="ops", name="ops")
                    for k2 in range(4):
                        nc.tensor.matmul(ps[0:cw, :],
                                         Ht[k2][:, 1 + 128 * c:1 + 128 * c + cw],
                                         owT[k2][:], start=(k2 == 0), stop=False)
                    nc.tensor.matmul(ps[0:cw, :], ones1[:, 0:cw], obT[:],
                                     start=False, stop=True)
                    negm = osb.tile([128, 1], dt.float32, tag="negm", name="negm")
                    nc.vector.tensor_reduce(negm[0:cw, :], ps[0:cw, :],
                                            axis=mybir.AxisListType.X,
                                            op=ALU.max, negate=True)
                    e = osb.tile([128, PHONE], dt.float32, tag="e", name="e")
                    ssum = osb.tile([128, 1], dt.float32, tag="ssum", name="ssum")
                    nc.scalar.activation(e[0:cw, :], ps[0:cw, :], AF.Exp, bias=negm[0:cw, :],
                                         scale=1.0, accum_out=ssum[0:cw, :])
                    ls = osb.tile([128, 1], dt.float32, tag="ls", name="ls")
                    nc.scalar.activation(ls[0:cw, :], ssum[0:cw, :], AF.Ln)
                    adj = osb.tile([128, 1], dt.float32, tag="adj", name="adj")
                    nc.vector.tensor_sub(adj[0:cw, :], negm[0:cw, :], ls[0:cw, :])
                    res = osb.tile([128, PHONE], dt.float32, tag="res", name="res")
                    nc.vector.tensor_scalar_add(res[0:cw, :], ps[0:cw, :], adj[0:cw, :])
                    nc.sync.dma_start(out[128 * c:128 * c + cw, :], res[0:cw, :])

    nc.compile()
    return nc


def _get_nc():
    if "nc" not in _cache:
        _cache["nc"] = _build_nc()
    return _cache["nc"]


def kernel(input_seq, h0, c0, conv_w, conv_b, w_ih, w_hh, b_ih, b_hh, out_w, out_b):
    from concourse.bass_utils import run_bass_kernel_spmd

    input_seq = np.asarray(input_seq, np.float32)
    shared = _host_pack(np.asarray(conv_w, np.float32), np.asarray(conv_b, np.float32),
                        np.asarray(w_ih, np.float32), np.asarray(w_hh, np.float32),
                        np.asarray(b_ih, np.float32), np.asarray(b_hh, np.float32),
                        np.asarray(out_w, np.float32), np.asarray(out_b, np.float32))

    # edge-padded input: rows -4-OV .. T+128+4 relative indexing via clip
    def in_slice(j):
        lo = j * BLK - OV - 4 if j > 0 else -4
        idx = np.clip(np.arange(lo, lo + LIN), 0, T - 1)
        return input_seq[idx]

    in_maps = []
    for j in range(NCORES):
        m = dict(shared)
        m["inp"] = in_slice(j)
        hcol = np.zeros((128, 8), np.float32)
        if j == 0:
            hcol[:, 0:4] = np.asarray(h0, np.float32).reshape(4, 128).T
            hcol[:, 4:8] = np.asarray(c0, np.float32).reshape(4, 128).T
        m["h0c0"] = hcol
        in_maps.append(m)

    nc = _get_nc()
    res = run_bass_kernel_spmd(nc, in_maps, list(range(NCORES)))

    outp = np.empty((T, PHONE), np.float32)
    for j in range(NCORES):
        o = res.results[j]["out"]
        if j == 0:
            outp[0:BLK] = o[0:BLK]
        else:
            outp[j * BLK:(j + 1) * BLK] = o[OV:OV + BLK]
    return outp



# revision 3
# speedup vs baseline: 1.3869x; 1.3869x over previous
"""Trainium2 Bass kernel: conv/pool front-end + LSTM + log_softmax.

Strategy (8 NeuronCores, no cross-core communication):
  - Time-shard T=8192 into 8 blocks of 1024; each core computes a
    1088-row window (64-row warm-up prefix discarded on the host).
  - Jacobi fixed-point iteration for the LSTM: 4 sweeps total (sweep 0
    from gates_x only, then 3 matmul sweeps). The recurrent matmuls run
    in fp8(e4m3) DoubleRow perf mode (2 contraction tiles per pass,
    0.5 cy/row) with weights pre-scaled by 64 (to stay in fp8 normals)
    and a matching 64*I identity matmul adding gates_x; the gate
    activation rescales by 1/64. The cell recurrence is solved exactly
    per sweep by the DVE prefix scan.
  - Gate bias is folded into gates_x via a ones-row in the B operand.
  - Conv front-end repacked into 5 stacked 128-row contraction tiles
    (5 matmuls per output chunk instead of 9); maxpool on GpSimd.
  - Gate activations are quad-merged: one ACT instruction covers the 4
    128-row blocks of a gate via a 4-bank PSUM tile.
"""

import numpy as np
import ml_dtypes

T = 8192
D = 106
H = 512
PHONE = 48
NCORES = 8
BLK = 1024          # rows owned per core
OV = 64             # warm-up prefix rows
L = BLK + OV        # 1088 rows computed per core
LIN = L + 8         # input rows incl. conv halo (+-4)
NSWEEPS = 4
FSC = 64.0          # fp8 weight prescale
NCH = [(0, 384), (384, 768), (768, 1088)]  # time chunks (free dim)
PSW = 384

bf16 = ml_dtypes.bfloat16
f8 = ml_dtypes.float8_e4m3

_cache = {}

# conv contraction stacking: rows r = dh*67 + f, 5 stacks of <=128 rows
STACKS = []
_r0 = 0
for _j in range(5):
    end = min(603, _r0 + 128)
    segs = []
    r = _r0
    while r < end:
        dh, f0 = divmod(r, 67)
        take = min(end - r, 67 - f0)
        segs.append((r - _r0, dh, f0, take))  # dst row, dh shift, src row, n
        r += take
    STACKS.append((_r0, end - _r0, segs))
    _r0 = end


def _host_pack(conv_w, conv_b, w_ih, w_hh, b_ih, b_hh, out_w, out_b):
    key = hash((conv_w.tobytes(), w_ih.tobytes(), w_hh.tobytes(), b_ih.tobytes(),
                b_hh.tobytes(), out_w.tobytes(), out_b.tobytes(), conv_b.tobytes()))
    if _cache.get("pack_key") == key:
        return _cache["pack"]
    # conv weights: col of i-chunk = 128i + 32d + j for pair p=32i+j, delta d
    WA = np.zeros((9, 67, 896), np.float32)
    p_all = np.arange(210)
    c_all, wp_all = np.divmod(p_all, 21)
    i_all, j_all = np.divmod(p_all, 32)
    for d in range(3):
        w_all = 3 * wp_all + d
        col = 128 * i_all + 32 * d + j_all
        for dv in range(5):
            WA[:, w_all + dv, col] = conv_w[c_all, 0, :, dv].T
    W5 = WA.reshape(603, 896)
    cb = np.repeat(conv_b, 21)
    beff = b_ih + b_hh + w_ih[:, :210] @ cb
    wihB = np.concatenate(
        [w_ih[:, 128:210].T, w_ih[:, 210:249].T, beff[None, :]], 0)  # [122,2048]
    pack = {
        "convW5": W5.astype(bf16),
        "wihA": w_ih[:, 0:128].T.copy().astype(bf16),
        "wihB": wihB.astype(bf16),
        "whh8": np.ascontiguousarray(
            (FSC * w_hh.T).reshape(4, 128, 2048).transpose(1, 0, 2)).astype(f8),
        "owT": np.ascontiguousarray(
            out_w.T.reshape(4, 128, PHONE).transpose(1, 0, 2)).astype(bf16),
        "outb": out_b.reshape(1, PHONE).astype(bf16),
        "identb": np.eye(128, dtype=np.float32).astype(bf16),
        "idS": (FSC * np.eye(128, dtype=np.float32)).astype(bf16),
    }
    _cache["pack_key"] = key
    _cache["pack"] = pack
    return pack


def _build_nc():
    import concourse.bacc as bacc
    import concourse.tile as tile
    import concourse.mybir as mybir

    dt = mybir.dt
    AF = mybir.ActivationFunctionType
    ALU = mybir.AluOpType
    DR = mybir.MatmulPerfMode.DoubleRow

    nc = bacc.Bacc(None, target_bir_lowering=False)

    inp = nc.declare_dram_parameter("inp", [LIN, D], dt.bfloat16, isOutput=False)
    h0c0 = nc.declare_dram_parameter("h0c0", [128, 8], dt.float32, isOutput=False)
    convW5 = nc.declare_dram_parameter("convW5", [603, 896], dt.bfloat16, isOutput=False)
    wihA = nc.declare_dram_parameter("wihA", [128, 2048], dt.bfloat16, isOutput=False)
    wihB = nc.declare_dram_parameter("wihB", [122, 2048], dt.bfloat16, isOutput=False)
    whh8 = nc.declare_dram_parameter("whh8", [128, 4, 2048], dt.float8e4, isOutput=False)
    owT = nc.declare_dram_parameter("owT", [128, 4, PHONE], dt.bfloat16, isOutput=False)
    outb = nc.declare_dram_parameter("outb", [1, PHONE], dt.bfloat16, isOutput=False)
    identb = nc.declare_dram_parameter("identb", [128, 128], dt.bfloat16, isOutput=False)
    idSp = nc.declare_dram_parameter("idS", [128, 128], dt.bfloat16, isOutput=False)
    out = nc.declare_dram_parameter("out", [L, PHONE], dt.float32, isOutput=True)

    SIG, TANH = AF.Sigmoid, AF.Tanh
    TAU_FUNC = [SIG, SIG, TANH, SIG]  # i, f, g, o

    with tile.TileContext(nc) as tc:
        with tc.tile_pool(name="persist", bufs=1) as pp:
            featT = pp.tile([67, LIN], dt.bfloat16, tag="featT", name="featT")
            S5 = [pp.tile([STACKS[j][1], L], dt.bfloat16, tag=f"S{j}", name=f"S{j}")
                  for j in range(5)]
            tileA = pp.tile([128, L], dt.bfloat16, tag="tileA", name="tileA")
            tileB = pp.tile([122, L], dt.bfloat16, tag="tileB", name="tileB")
            gxall = pp.tile([128, 16, L], dt.bfloat16, tag="gxall", name="gxall")
            H8 = pp.tile([128, 4, L + 1], dt.float8e4, tag="H8", name="H8")
            Hb = pp.tile([128, 4, L], dt.bfloat16, tag="Hb", name="Hb")
            Ct = pp.tile([128, 4, L], dt.float32, tag="Ct", name="Ct")
            W5t = [pp.tile([STACKS[j][1], 896], dt.bfloat16, tag=f"W5_{j}",
                           name=f"W5_{j}") for j in range(5)]
            wiA = pp.tile([128, 2048], dt.bfloat16, tag="wiA", name="wiA")
            wiB = pp.tile([122, 2048], dt.bfloat16, tag="wiB", name="wiB")
            w8 = pp.tile([128, 4, 2048], dt.float8e4, tag="w8", name="w8")
            ow = pp.tile([128, 4, PHONE], dt.bfloat16, tag="ow", name="ow")
            ob = pp.tile([1, PHONE], dt.bfloat16, tag="ob", name="ob")
            idb = pp.tile([128, 128], dt.bfloat16, tag="idb", name="idb")
            idS = pp.tile([128, 128], dt.bfloat16, tag="idS", name="idS")
            hc = pp.tile([128, 8], dt.float32, tag="hc", name="hc")
            ones1 = pp.tile([1, 128], dt.bfloat16, tag="ones1", name="ones1")

            # weight DMAs: spread across scalar/gpsimd queues; input stays on sync
            _dmas = ([(w8, whh8), (wiA, wihA), (wiB, wihB), (ow, owT), (ob, outb),
                      (idb, identb), (idS, idSp), (hc, h0c0)]
                     + [(W5t[j], convW5[STACKS[j][0]:STACKS[j][0] + STACKS[j][1], :])
                        for j in range(5)])
            for _i, (dst, src) in enumerate(_dmas):
                (nc.scalar if _i % 2 else nc.gpsimd).dma_start(dst[:], src[:])
            nc.gpsimd.memset(ones1[:], 1.0)
            nc.gpsimd.memset(tileB[121:122, :], 1.0)
            # h0 -> fp8 H boundary col
            nc.vector.tensor_copy(H8[:, :, 0:1], hc[:, 0:4])

            # ---- input DMA + transpose (chunks of <=122 rows) ----
            tchunks = []
            _p = 0
            while _p < LIN:
                _w = min(122, LIN - _p)
                tchunks.append((_p, _w))
                _p += _w
            with tc.tile_pool(name="tp_in", bufs=3) as tin, \
                 tc.tile_pool(name="tp_ps", bufs=2, space="PSUM") as tps:
                for (p0, cw) in tchunks:
                    xt = tin.tile([122, D], dt.bfloat16, tag="xt", name="xt")
                    nc.sync.dma_start(xt[0:cw, :], inp[p0:p0 + cw, :])
                    pt = tps.tile([D, 122], dt.bfloat16, tag="pt", name="pt")
                    nc.tensor.transpose(pt[:, 0:cw], xt[0:cw, :], idb[0:cw, 0:cw])
                    nc.vector.tensor_copy(featT[:, p0:p0 + cw], pt[39:106, 0:cw])
                    # mfcc -> tileB rows 82..120, shifted by -4 (halo offset)
                    d0 = p0 - 4
                    s0, s1 = max(0, -d0), min(cw, L - d0)
                    if s1 > s0:
                        nc.vector.tensor_copy(
                            tileB[82:121, d0 + s0:d0 + s1], pt[0:39, s0:s1])

            # ---- conv contraction stacks ----
            for j in range(5):
                for (dr, dh, f0, n) in STACKS[j][2]:
                    nc.vector.tensor_copy(S5[j][dr:dr + n, :],
                                          featT[f0:f0 + n, dh:dh + L])

            # ---- conv + maxpool ----
            with tc.tile_pool(name="cv_ps", bufs=3, space="PSUM") as cps, \
                 tc.tile_pool(name="cv_sb", bufs=3) as csb:
                for i in range(7):
                    for (n0, n1) in NCH:
                        w = n1 - n0
                        ps = cps.tile([128, PSW], dt.float32, tag="cvps", name="cvps")
                        for j in range(5):
                            nc.tensor.matmul(
                                ps[:, 0:w],
                                W5t[j][:, 128 * i:128 * (i + 1)],
                                S5[j][:, n0:n1],
                                start=(j == 0), stop=(j == 4))
                        if i < 4:
                            dst = tileA[32 * i:32 * (i + 1), n0:n1]
                            rows = 32
                        elif i < 6:
                            dst = tileB[32 * (i - 4):32 * (i - 3), n0:n1]
                            rows = 32
                        else:
                            dst = tileB[64:82, n0:n1]
                            rows = 18
                        c96 = csb.tile([96, PSW], dt.bfloat16, tag="c96", name="c96")
                        t32 = csb.tile([32, PSW], dt.bfloat16, tag="t32", name="t32")
                        nc.vector.tensor_copy(c96[0:64 + rows, 0:w],
                                              ps[0:64 + rows, 0:w])
                        nc.gpsimd.tensor_max(t32[0:rows, 0:w], c96[0:rows, 0:w],
                                             c96[32:32 + rows, 0:w])
                        nc.gpsimd.tensor_max(dst, t32[0:rows, 0:w],
                                             c96[64:64 + rows, 0:w])

            # ---- gates_x (incl. bias via ones row) ----
            with tc.tile_pool(name="gx_ps", bufs=4, space="PSUM") as gps:
                for m in range(16):
                    for (n0, n1) in NCH:
                        w = n1 - n0
                        ps = gps.tile([128, PSW], dt.float32, tag="gxps", name="gxps")
                        nc.tensor.matmul(ps[:, 0:w], wiA[:, 128 * m:128 * (m + 1)],
                                         tileA[:, n0:n1], start=True, stop=False)
                        nc.tensor.matmul(ps[:, 0:w], wiB[:, 128 * m:128 * (m + 1)],
                                         tileB[:, n0:n1], start=False, stop=True)
                        nc.vector.tensor_copy(gxall[:, m, n0:n1], ps[:, 0:w])

            # ---- Jacobi sweeps ----
            with tc.tile_pool(name="sw_ps", bufs=2, space="PSUM") as sps, \
                 tc.tile_pool(name="sw_sb", bufs=2) as ssb:
                for s in range(NSWEEPS):
                    last = (s == NSWEEPS - 1)
                    for (n0, n1) in NCH:
                        w = n1 - n0
                        gq = {}
                        # gate order: f, i, g first (scan inputs), then o
                        for tau in (1, 0, 2, 3):
                            g = ssb.tile([128, 4, PSW], dt.bfloat16,
                                         tag=f"g{tau}", name=f"g{tau}")
                            if s == 0:
                                nc.scalar.activation(
                                    g[:, :, 0:w], gxall[:, 4 * tau:4 * tau + 4, n0:n1],
                                    TAU_FUNC[tau])
                            else:
                                qp = sps.tile([128, 4, 512], dt.float32,
                                              tag="qp", name="qp")
                                for m4 in range(4):
                                    m = 4 * tau + m4
                                    nc.tensor.matmul(
                                        qp[:, m4, 0:w], w8[:, 0:2, 128 * m:128 * (m + 1)],
                                        H8[:, 0:2, n0:n1],
                                        start=True, stop=False, perf_mode=DR)
                                    nc.tensor.matmul(
                                        qp[:, m4, 0:w], w8[:, 2:4, 128 * m:128 * (m + 1)],
                                        H8[:, 2:4, n0:n1],
                                        start=False, stop=False, perf_mode=DR)
                                    nc.tensor.matmul(
                                        qp[:, m4, 0:w], idS[:],
                                        gxall[:, m, n0:n1], start=False, stop=True)
                                nc.scalar.activation(
                                    g[:, :, 0:w], qp[:, :, 0:w], TAU_FUNC[tau],
                                    scale=1.0 / FSC)
                            gq[tau] = g
                        uq = ssb.tile([128, 4, PSW], dt.bfloat16, tag="uq", name="uq")
                        nc.vector.tensor_mul(uq[:, :, 0:w], gq[0][:, :, 0:w],
                                             gq[2][:, :, 0:w])
                        for k in range(4):
                            init = hc[:, 4 + k:5 + k] if n0 == 0 else Ct[:, k, n0 - 1:n0]
                            nc.vector.tensor_tensor_scan(
                                Ct[:, k, n0:n1], gq[1][:, k, 0:w], uq[:, k, 0:w],
                                init, ALU.mult, ALU.add)
                        tcq = ssb.tile([128, 4, PSW], dt.bfloat16, tag="tcq", name="tcq")
                        nc.scalar.activation(tcq[:, :, 0:w], Ct[:, :, n0:n1], TANH)
                        if last:
                            nc.vector.tensor_mul(Hb[:, :, n0:n1], gq[3][:, :, 0:w],
                                                 tcq[:, :, 0:w])
                        else:
                            nc.vector.tensor_mul(H8[:, :, 1 + n0:1 + n1],
                                                 gq[3][:, :, 0:w], tcq[:, :, 0:w])

            # ---- output projection + log_softmax ----
            with tc.tile_pool(name="o_ps", bufs=3, space="PSUM") as ops, \
                 tc.tile_pool(name="o_sb", bufs=3) as osb:
                for c in range((L + 127) // 128):
                    cw = min(128, L - 128 * c)
                    ps = ops.tile([128, PHONE], dt.float32, tag="ops", name="ops")
                    for k in range(4):
                        nc.tensor.matmul(ps[0:cw, :],
                                         Hb[:, k, 128 * c:128 * c + cw],
                                         ow[:, k, :], start=(k == 0), stop=False)
                    nc.tensor.matmul(ps[0:cw, :], ones1[:, 0:cw], ob[:],
                                     start=False, stop=True)
                    negm = osb.tile([128, 1], dt.float32, tag="negm", name="negm")
                    nc.vector.tensor_reduce(negm[0:cw, :], ps[0:cw, :],
                                            axis=mybir.AxisListType.X,
                                            op=ALU.max, negate=True)
                    e = osb.tile([128, PHONE], dt.float32, tag="e", name="e")
                    ssum = osb.tile([128, 1], dt.float32, tag="ssum", name="ssum")
                    nc.scalar.activation(e[0:cw, :], ps[0:cw, :], AF.Exp,
                                         bias=negm[0:cw, :], scale=1.0,
                                         accum_out=ssum[0:cw, :])
                    ls = osb.tile([128, 1], dt.float32, tag="ls", name="ls")
                    nc.scalar.activation(ls[0:cw, :], ssum[0:cw, :], AF.Ln)
                    adj = osb.tile([128, 1], dt.float32, tag="adj", name="adj")
                    nc.vector.tensor_sub(adj[0:cw, :], negm[0:cw, :], ls[0:cw, :])
                    res = osb.tile([128, PHONE], dt.float32, tag="res", name="res")
                    nc.vector.tensor_scalar_add(res[0:cw, :], ps[0:cw, :], adj[0:cw, :])
                    nc.sync.dma_start(out[128 * c:128 * c + cw, :], res[0:cw, :])

    nc.compile()
    return nc


def _get_nc():
    if "nc" not in _cache:
        _cache["nc"] = _build_nc()
    return _cache["nc"]


def kernel(input_seq, h0, c0, conv_w, conv_b, w_ih, w_hh, b_ih, b_hh, out_w, out_b):
    from concourse.bass_utils import run_bass_kernel_spmd

    input_seq = np.asarray(input_seq, np.float32)
    shared = _host_pack(np.asarray(conv_w, np.float32), np.asarray(conv_b, np.float32),
                        np.asarray(w_ih, np.float32), np.asarray(w_hh, np.float32),
                        np.asarray(b_ih, np.float32), np.asarray(b_hh, np.float32),
                        np.asarray(out_w, np.float32), np.asarray(out_b, np.float32))

    def in_slice(j):
        lo = j * BLK - OV - 4 if j > 0 else -4
        idx = np.clip(np.arange(lo, lo + LIN), 0, T - 1)
        return input_seq[idx].astype(bf16)

    in_maps = []
    for j in range(NCORES):
        m = dict(shared)
        m["inp"] = in_slice(j)
        hcol = np.zeros((128, 8), np.float32)
        if j == 0:
            hcol[:, 0:4] = np.asarray(h0, np.float32).reshape(4, 128).T
            hcol[:, 4:8] = np.asarray(c0, np.float32).reshape(4, 128).T
        m["h0c0"] = hcol
        in_maps.append(m)

    nc = _get_nc()
    res = run_bass_kernel_spmd(nc, in_maps, list(range(NCORES)))

    outp = np.empty((T, PHONE), np.float32)
    for j in range(NCORES):
        o = res.results[j]["out"]
        if j == 0:
            outp[0:BLK] = o[0:BLK]
        else:
            outp[j * BLK:(j + 1) * BLK] = o[OV:OV + BLK]
    return outp


# revision 7
# speedup vs baseline: 1.7521x; 1.2633x over previous
"""Trainium2 Bass kernel: conv/pool front-end + LSTM + log_softmax.

Strategy (8 NeuronCores, no cross-core communication):
  - Time-shard T=8192 into 8 blocks of 1024; each core computes a
    1088-row window (64-row warm-up prefix discarded on the host).
  - Jacobi fixed-point iteration for the LSTM: 4 sweeps total (sweep 0
    from gates_x only, then 3 matmul sweeps). The recurrent matmuls run
    in fp8(e4m3) DoubleRow perf mode (2 contraction tiles per pass,
    0.5 cy/row) with weights pre-scaled by 64 (to stay in fp8 normals)
    and a matching 64*I identity matmul adding gates_x; the gate
    activation rescales by 1/64. The cell recurrence is solved exactly
    per sweep by the DVE prefix scan.
  - Gate bias is folded into gates_x via a ones-row in the B operand.
  - Conv front-end repacked into 5 stacked 128-row contraction tiles
    (5 matmuls per output chunk instead of 9); maxpool on GpSimd.
  - Gate activations are quad-merged: one ACT instruction covers the 4
    128-row blocks of a gate via a 4-bank PSUM tile.
"""

import numpy as np
import ml_dtypes

T = 8192
D = 106
H = 512
PHONE = 48
NCORES = 8
BLK = 1024          # rows owned per core
OV = 64             # warm-up prefix rows
L = BLK + OV        # 1088 rows computed per core
LIN = L + 8         # input rows incl. conv halo (+-4)
NSWEEPS = 4
FSC = 64.0          # fp8 weight prescale
NCH = [(0, 384), (384, 768), (768, 1088)]  # time chunks (free dim)
PSW = 384

bf16 = ml_dtypes.bfloat16
f8 = ml_dtypes.float8_e4m3

_cache = {}

# conv contraction stacking: rows r = dh*67 + f, 5 stacks of <=128 rows
STACKS = []
_r0 = 0
for _j in range(5):
    end = min(603, _r0 + 128)
    segs = []
    r = _r0
    while r < end:
        dh, f0 = divmod(r, 67)
        take = min(end - r, 67 - f0)
        segs.append((r - _r0, dh, f0, take))  # dst row, dh shift, src row, n
        r += take
    STACKS.append((_r0, end - _r0, segs))
    _r0 = end


def _host_pack(conv_w, conv_b, w_ih, w_hh, b_ih, b_hh, out_w, out_b):
    key = hash((conv_w.tobytes(), w_ih.tobytes(), w_hh.tobytes(), b_ih.tobytes(),
                b_hh.tobytes(), out_w.tobytes(), out_b.tobytes(), conv_b.tobytes()))
    if _cache.get("pack_key") == key:
        return _cache["pack"]
    # conv weights: col of i-chunk = 128i + 32d + j for pair p=32i+j, delta d
    WA = np.zeros((9, 67, 896), np.float32)
    p_all = np.arange(210)
    c_all, wp_all = np.divmod(p_all, 21)
    i_all, j_all = np.divmod(p_all, 32)
    for d in range(3):
        w_all = 3 * wp_all + d
        col = 128 * i_all + 32 * d + j_all
        for dv in range(5):
            WA[:, w_all + dv, col] = conv_w[c_all, 0, :, dv].T
    W5 = WA.reshape(603, 896)
    cb = np.repeat(conv_b, 21)
    beff = b_ih + b_hh + w_ih[:, :210] @ cb
    wihB = np.concatenate(
        [w_ih[:, 128:210].T, w_ih[:, 210:249].T, beff[None, :]], 0)  # [122,2048]
    pack = {
        "convW5": W5.astype(bf16),
        "wihA": w_ih[:, 0:128].T.copy().astype(bf16),
        "wihB": wihB.astype(bf16),
        "whh8": np.ascontiguousarray(
            (FSC * w_hh.T).reshape(4, 128, 2048).transpose(1, 0, 2)).astype(f8),
        "owT": np.ascontiguousarray(
            out_w.T.reshape(4, 128, PHONE).transpose(1, 0, 2)).astype(bf16),
        "outb": out_b.reshape(1, PHONE).astype(bf16),
        "identb": np.eye(128, dtype=np.float32).astype(bf16),
        "idS": (FSC * np.eye(128, dtype=np.float32)).astype(bf16),
    }
    _cache["pack_key"] = key
    _cache["pack"] = pack
    return pack


def _build_nc():
    import concourse.bacc as bacc
    import concourse.tile as tile
    import concourse.mybir as mybir

    dt = mybir.dt
    AF = mybir.ActivationFunctionType
    ALU = mybir.AluOpType
    DR = mybir.MatmulPerfMode.DoubleRow

    nc = bacc.Bacc(None, target_bir_lowering=False)

    inp = nc.declare_dram_parameter("inp", [LIN, D], dt.bfloat16, isOutput=False)
    h0c0 = nc.declare_dram_parameter("h0c0", [128, 8], dt.float32, isOutput=False)
    convW5 = nc.declare_dram_parameter("convW5", [603, 896], dt.bfloat16, isOutput=False)
    wihA = nc.declare_dram_parameter("wihA", [128, 2048], dt.bfloat16, isOutput=False)
    wihB = nc.declare_dram_parameter("wihB", [122, 2048], dt.bfloat16, isOutput=False)
    whh8 = nc.declare_dram_parameter("whh8", [128, 4, 2048], dt.float8e4, isOutput=False)
    owT = nc.declare_dram_parameter("owT", [128, 4, PHONE], dt.bfloat16, isOutput=False)
    outb = nc.declare_dram_parameter("outb", [1, PHONE], dt.bfloat16, isOutput=False)
    identb = nc.declare_dram_parameter("identb", [128, 128], dt.bfloat16, isOutput=False)
    idSp = nc.declare_dram_parameter("idS", [128, 128], dt.bfloat16, isOutput=False)
    out = nc.declare_dram_parameter("out", [L, PHONE], dt.float32, isOutput=True)

    SIG, TANH = AF.Sigmoid, AF.Tanh
    TAU_FUNC = [SIG, SIG, TANH, SIG]  # i, f, g, o

    with tile.TileContext(nc) as tc:
        with tc.tile_pool(name="persist", bufs=1) as pp:
            featT = pp.tile([67, LIN], dt.bfloat16, tag="featT", name="featT")
            S5 = [pp.tile([STACKS[j][1], L], dt.bfloat16, tag=f"S{j}", name=f"S{j}")
                  for j in range(5)]
            tileA = pp.tile([128, L], dt.bfloat16, tag="tileA", name="tileA")
            tileB = pp.tile([122, L], dt.bfloat16, tag="tileB", name="tileB")
            gxall = pp.tile([128, 16, L], dt.bfloat16, tag="gxall", name="gxall")
            # double-buffered fp8 H (pure Jacobi: no cross-chunk write/read
            # overlap within a sweep)
            H8 = [pp.tile([128, 4, L + 1], dt.float8e4, tag=f"H8{i}",
                          name=f"H8{i}") for i in range(2)]
            Hb = pp.tile([128, 4, L], dt.bfloat16, tag="Hb", name="Hb")
            Ct = pp.tile([128, 4, L], dt.float32, tag="Ct", name="Ct")
            W5t = [pp.tile([STACKS[j][1], 896], dt.bfloat16, tag=f"W5_{j}",
                           name=f"W5_{j}") for j in range(5)]
            wiA = pp.tile([128, 2048], dt.bfloat16, tag="wiA", name="wiA")
            wiB = pp.tile([122, 2048], dt.bfloat16, tag="wiB", name="wiB")
            w8 = pp.tile([128, 4, 2048], dt.float8e4, tag="w8", name="w8")
            ow = pp.tile([128, 4, PHONE], dt.bfloat16, tag="ow", name="ow")
            ob = pp.tile([1, PHONE], dt.bfloat16, tag="ob", name="ob")
            idb = pp.tile([128, 128], dt.bfloat16, tag="idb", name="idb")
            idS = pp.tile([128, 128], dt.bfloat16, tag="idS", name="idS")
            hc = pp.tile([128, 8], dt.float32, tag="hc", name="hc")
            ones1 = pp.tile([1, 128], dt.bfloat16, tag="ones1", name="ones1")

            # weight DMAs: spread across scalar/gpsimd queues; input stays on sync
            _dmas = ([(w8, whh8), (wiA, wihA), (wiB, wihB), (ow, owT), (ob, outb),
                      (idb, identb), (idS, idSp), (hc, h0c0)]
                     + [(W5t[j], convW5[STACKS[j][0]:STACKS[j][0] + STACKS[j][1], :])
                        for j in range(5)])
            for _i, (dst, src) in enumerate(_dmas):
                (nc.scalar if _i % 2 else nc.gpsimd).dma_start(dst[:], src[:])
            nc.gpsimd.memset(ones1[:], 1.0)
            nc.gpsimd.memset(tileB[121:122, :], 1.0)
            # h0 -> fp8 H boundary col (both buffers)
            nc.vector.tensor_copy(H8[0][:, :, 0:1], hc[:, 0:4])
            nc.vector.tensor_copy(H8[1][:, :, 0:1], hc[:, 0:4])

            # ---- input DMA + transpose (chunks of <=122 rows) ----
            tchunks = []
            _p = 0
            while _p < LIN:
                _w = min(122, LIN - _p)
                tchunks.append((_p, _w))
                _p += _w
            with tc.tile_pool(name="tp_in", bufs=3) as tin, \
                 tc.tile_pool(name="tp_ps", bufs=2, space="PSUM") as tps:
                for (p0, cw) in tchunks:
                    xt = tin.tile([122, D], dt.bfloat16, tag="xt", name="xt")
                    nc.sync.dma_start(xt[0:cw, :], inp[p0:p0 + cw, :])
                    pt = tps.tile([D, 122], dt.bfloat16, tag="pt", name="pt")
                    nc.tensor.transpose(pt[:, 0:cw], xt[0:cw, :], idb[0:cw, 0:cw])
                    nc.vector.tensor_copy(featT[:, p0:p0 + cw], pt[39:106, 0:cw])
                    # mfcc -> tileB rows 82..120, shifted by -4 (halo offset)
                    d0 = p0 - 4
                    s0, s1 = max(0, -d0), min(cw, L - d0)
                    if s1 > s0:
                        nc.vector.tensor_copy(
                            tileB[82:121, d0 + s0:d0 + s1], pt[0:39, s0:s1])

            # ---- conv + maxpool + gates_x, chunk-major for pipelining ----
            with tc.tile_pool(name="cv_ps", bufs=3, space="PSUM") as cps, \
                 tc.tile_pool(name="cv_sb", bufs=3) as csb, \
                 tc.tile_pool(name="gx_ps", bufs=4, space="PSUM") as gps:
                for (n0, n1) in NCH:
                    w = n1 - n0
                    # conv contraction stacks for this chunk
                    for j in range(5):
                        for (dr, dh, f0, n) in STACKS[j][2]:
                            nc.vector.tensor_copy(
                                S5[j][dr:dr + n, n0:n1],
                                featT[f0:f0 + n, n0 + dh:n1 + dh])
                    for i in range(7):
                        ps = cps.tile([128, PSW], dt.float32, tag="cvps", name="cvps")
                        for j in range(5):
                            nc.tensor.matmul(
                                ps[:, 0:w],
                                W5t[j][:, 128 * i:128 * (i + 1)],
                                S5[j][:, n0:n1],
                                start=(j == 0), stop=(j == 4))
                        if i < 4:
                            dst = tileA[32 * i:32 * (i + 1), n0:n1]
                            rows = 32
                        elif i < 6:
                            dst = tileB[32 * (i - 4):32 * (i - 3), n0:n1]
                            rows = 32
                        else:
                            dst = tileB[64:82, n0:n1]
                            rows = 18
                        c96 = csb.tile([96, PSW], dt.bfloat16, tag="c96", name="c96")
                        t32 = csb.tile([32, PSW], dt.bfloat16, tag="t32", name="t32")
                        nc.vector.tensor_copy(c96[0:64 + rows, 0:w],
                                              ps[0:64 + rows, 0:w])
                        nc.gpsimd.tensor_max(t32[0:rows, 0:w], c96[0:rows, 0:w],
                                             c96[32:32 + rows, 0:w])
                        nc.gpsimd.tensor_max(dst, t32[0:rows, 0:w],
                                             c96[64:64 + rows, 0:w])
                    # gates_x for this chunk (incl. bias via ones row)
                    for m in range(16):
                        ps = gps.tile([128, PSW], dt.float32, tag="gxps", name="gxps")
                        nc.tensor.matmul(ps[:, 0:w], wiA[:, 128 * m:128 * (m + 1)],
                                         tileA[:, n0:n1], start=True, stop=False)
                        nc.tensor.matmul(ps[:, 0:w], wiB[:, 128 * m:128 * (m + 1)],
                                         tileB[:, n0:n1], start=False, stop=True)
                        nc.vector.tensor_copy(gxall[:, m, n0:n1], ps[:, 0:w])

            # ---- Jacobi sweeps ----
            with tc.tile_pool(name="sw_ps", bufs=2, space="PSUM") as sps, \
                 tc.tile_pool(name="sw_sb", bufs=2) as ssb:
                for s in range(NSWEEPS):
                    last = (s == NSWEEPS - 1)
                    Hrd = H8[(s - 1) % 2]
                    Hwr = H8[s % 2]
                    for (n0, n1) in NCH:
                        w = n1 - n0
                        gq = {}
                        # gate order: f, i, g first (scan inputs), then o
                        for tau in (1, 0, 2, 3):
                            g = ssb.tile([128, 4, PSW], dt.bfloat16,
                                         tag=f"g{tau}", name=f"g{tau}")
                            if s == 0:
                                nc.scalar.activation(
                                    g[:, :, 0:w], gxall[:, 4 * tau:4 * tau + 4, n0:n1],
                                    TAU_FUNC[tau])
                            else:
                                qp = sps.tile([128, 4, 512], dt.float32,
                                              tag="qp", name="qp")
                                for m4 in range(4):
                                    m = 4 * tau + m4
                                    nc.tensor.matmul(
                                        qp[:, m4, 0:w], w8[:, 0:2, 128 * m:128 * (m + 1)],
                                        Hrd[:, 0:2, n0:n1],
                                        start=True, stop=False, perf_mode=DR)
                                    nc.tensor.matmul(
                                        qp[:, m4, 0:w], w8[:, 2:4, 128 * m:128 * (m + 1)],
                                        Hrd[:, 2:4, n0:n1],
                                        start=False, stop=False, perf_mode=DR)
                                    nc.tensor.matmul(
                                        qp[:, m4, 0:w], idS[:],
                                        gxall[:, m, n0:n1], start=False, stop=True)
                                nc.scalar.activation(
                                    g[:, :, 0:w], qp[:, :, 0:w], TAU_FUNC[tau],
                                    scale=1.0 / FSC)
                            gq[tau] = g
                        uq = ssb.tile([128, 4, PSW], dt.bfloat16, tag="uq", name="uq")
                        nc.vector.tensor_mul(uq[:, :, 0:w], gq[0][:, :, 0:w],
                                             gq[2][:, :, 0:w])
                        for k in range(4):
                            init = hc[:, 4 + k:5 + k] if n0 == 0 else Ct[:, k, n0 - 1:n0]
                            nc.vector.tensor_tensor_scan(
                                Ct[:, k, n0:n1], gq[1][:, k, 0:w], uq[:, k, 0:w],
                                init, ALU.mult, ALU.add)
                        tcq = ssb.tile([128, 4, PSW], dt.bfloat16, tag="tcq", name="tcq")
                        nc.scalar.activation(tcq[:, :, 0:w], Ct[:, :, n0:n1], TANH)
                        if last:
                            nc.vector.tensor_mul(Hb[:, :, n0:n1], gq[3][:, :, 0:w],
                                                 tcq[:, :, 0:w])
                        else:
                            # split the fp8 H write: DVE pair 0, Pool pair 1 —
                            # unblocks the next sweep's pair-0 matmuls sooner
                            nc.vector.tensor_mul(Hwr[:, 0:2, 1 + n0:1 + n1],
                                                 gq[3][:, 0:2, 0:w], tcq[:, 0:2, 0:w])
                            nc.gpsimd.tensor_mul(Hwr[:, 2:4, 1 + n0:1 + n1],
                                                 gq[3][:, 2:4, 0:w], tcq[:, 2:4, 0:w])

            # ---- output projection + log_softmax ----
            with tc.tile_pool(name="o_ps", bufs=3, space="PSUM") as ops, \
                 tc.tile_pool(name="o_sb", bufs=3) as osb:
                for c in range((L + 127) // 128):
                    cw = min(128, L - 128 * c)
                    ps = ops.tile([128, PHONE], dt.float32, tag="ops", name="ops")
                    for k in range(4):
                        nc.tensor.matmul(ps[0:cw, :],
                                         Hb[:, k, 128 * c:128 * c + cw],
                                         ow[:, k, :], start=(k == 0), stop=False)
                    nc.tensor.matmul(ps[0:cw, :], ones1[:, 0:cw], ob[:],
                                     start=False, stop=True)
                    negm = osb.tile([128, 1], dt.float32, tag="negm", name="negm")
                    nc.vector.tensor_reduce(negm[0:cw, :], ps[0:cw, :],
                                            axis=mybir.AxisListType.X,
                                            op=ALU.max, negate=True)
                    e = osb.tile([128, PHONE], dt.float32, tag="e", name="e")
                    ssum = osb.tile([128, 1], dt.float32, tag="ssum", name="ssum")
                    nc.scalar.activation(e[0:cw, :], ps[0:cw, :], AF.Exp,
                                         bias=negm[0:cw, :], scale=1.0,
                                         accum_out=ssum[0:cw, :])
                    ls = osb.tile([128, 1], dt.float32, tag="ls", name="ls")
                    nc.scalar.activation(ls[0:cw, :], ssum[0:cw, :], AF.Ln)
                    adj = osb.tile([128, 1], dt.float32, tag="adj", name="adj")
                    nc.vector.tensor_sub(adj[0:cw, :], negm[0:cw, :], ls[0:cw, :])
                    res = osb.tile([128, PHONE], dt.float32, tag="res", name="res")
                    nc.vector.tensor_scalar_add(res[0:cw, :], ps[0:cw, :], adj[0:cw, :])
                    nc.sync.dma_start(out[128 * c:128 * c + cw, :], res[0:cw, :])

    nc.compile()
    return nc


def _get_nc():
    if "nc" not in _cache:
        _cache["nc"] = _build_nc()
    return _cache["nc"]


def kernel(input_seq, h0, c0, conv_w, conv_b, w_ih, w_hh, b_ih, b_hh, out_w, out_b):
    from concourse.bass_utils import run_bass_kernel_spmd

    input_seq = np.asarray(input_seq, np.float32)
    shared = _host_pack(np.asarray(conv_w, np.float32), np.asarray(conv_b, np.float32),
                        np.asarray(w_ih, np.float32), np.asarray(w_hh, np.float32),
                        np.asarray(b_ih, np.float32), np.asarray(b_hh, np.float32),
                        np.asarray(out_w, np.float32), np.asarray(out_b, np.float32))

    def in_slice(j):
        lo = j * BLK - OV - 4 if j > 0 else -4
        idx = np.clip(np.arange(lo, lo + LIN), 0, T - 1)
        return input_seq[idx].astype(bf16)

    in_maps = []
    for j in range(NCORES):
        m = dict(shared)
        m["inp"] = in_slice(j)
        hcol = np.zeros((128, 8), np.float32)
        if j == 0:
            hcol[:, 0:4] = np.asarray(h0, np.float32).reshape(4, 128).T
            hcol[:, 4:8] = np.asarray(c0, np.float32).reshape(4, 128).T
        m["h0c0"] = hcol
        in_maps.append(m)

    nc = _get_nc()
    res = run_bass_kernel_spmd(nc, in_maps, list(range(NCORES)))

    outp = np.empty((T, PHONE), np.float32)
    for j in range(NCORES):
        o = res.results[j]["out"]
        if j == 0:
            outp[0:BLK] = o[0:BLK]
        else:
            outp[j * BLK:(j + 1) * BLK] = o[OV:OV + BLK]
    return outp
